# revision 1
# baseline (speedup 1.0000x reference)
"""Trainium2 Bass kernel for nn_MultiHeadCrossAttention_47519518163418.

Sharding: 8 cores = (batch b in {0,1}) x (head h in {0..3}); core c: b=c//4, h=c%4.
Each core computes q/k/v for its head's 32 channels (conv output channels are
independent), runs the full 4096x4096 attention for that head (flash-style,
scores computed transposed so no transposes of the score matrix are needed,
softmax without max-subtraction, row sums via an appended ones-column in the
PV matmul), then the cores of a batch AllGather the attention output to form
the full 128-channel mha2d. Green (upsample+conv+ILN+silu) and purple
(upsample+conv+ILN+sigmoid, gated by s) paths are computed per-core for the
core's 32 output channels using a phase-collapsed 2x2-tap decomposition of
"upsample2 + reflect-pad + 3x3 conv" (which reduces to edge-clamp padding on
the original-resolution image). ILN layer statistics are combined with one
tiny AllReduce. Host precomputes positional encodings + paddings and
reassembles the sharded outputs.
"""

import sys

if "/opt/trn_rl_repo" not in sys.path:
    sys.path.insert(0, "/opt/trn_rl_repo")

import numpy as np

NUM_HEADS = 4
EPS = 1e-5
D_HEAD = 32
SCALE = float(D_HEAD) ** -0.5
N_PX = 16384.0          # pixels per channel of the upsampled image
N_TOT = 128 * 16384.0   # elements per batch for layer stats

_CORES = list(range(8))
_REPLICA_GROUPS = [[0, 1, 2, 3], [4, 5, 6, 7]]


# ----------------------------------------------------------------------------
# Host-side helpers
# ----------------------------------------------------------------------------

def pos_encoding_pe(c, L, dtype=np.float32):
    half = c // 2
    pos = np.arange(L, dtype=dtype)
    depths = np.arange(half, dtype=dtype) / half
    rates = 1.0 / (10000.0 ** depths)
    ang = pos[:, None] * rates[None, :]
    pe = np.concatenate([np.sin(ang), np.cos(ang)], axis=-1)  # [L, c]
    return pe.T.astype(dtype)  # [c, L]


def reflect_pad(x):
    return np.pad(x, ((0, 0), (1, 1), (1, 1)), mode="reflect")


def edge_pad(x):
    return np.pad(x, ((0, 0), (1, 1), (1, 1)), mode="edge")


_KSET = {(0, 0): [0], (0, 1): [1, 2], (1, 0): [0, 1], (1, 1): [2]}


def collapse_w2(w):
    """w [co, ci, 3, 3] -> W2 [4 (p=2*pr+pc), 2 (dy), 2 (dx), ci, co]."""
    co, ci = w.shape[0], w.shape[1]
    W2 = np.zeros((4, 2, 2, ci, co), dtype=w.dtype)
    for pr in range(2):
        for pc in range(2):
            p = 2 * pr + pc
            for dy in range(2):
                for dx in range(2):
                    acc = np.zeros((co, ci), dtype=np.float64)
                    for ky in _KSET[(pr, dy)]:
                        for kx in _KSET[(pc, dx)]:
                            acc = acc + w[:, :, ky, kx].astype(np.float64)
                    W2[p, dy, dx] = acc.T.astype(w.dtype)
    return W2


def arrange_to_strips(x2d):
    """x [32, 128, 128] -> arranged [128, 4096] phase-major: partition
    32*(2*pr+pc)+c, free r*64+cc for upsampled pixel (2r+pr, 2cc+pc)."""
    t = x2d.reshape(32, 64, 2, 64, 2)          # c, r, pr, cc, pc
    t = t.transpose(2, 4, 0, 1, 3)              # pr, pc, c, r, cc
    return np.ascontiguousarray(t.reshape(128, 4096))


def unarrange_from_strips(arr):
    t = arr.reshape(2, 2, 32, 64, 64)           # pr, pc, c, r, cc
    t = t.transpose(2, 3, 0, 4, 1)              # c, r, pr, cc, pc
    return np.ascontiguousarray(t.reshape(32, 128, 128))


_PE_Y = None
_PE_S = None


_BATCH_CACHE = {}


def _batch_shared(inputs, b):
    """Padded/PE-added tensors shared by the 4 cores of a batch."""
    key = (id(inputs), b)
    if key in _BATCH_CACHE:
        return _BATCH_CACHE[key]
    y = np.asarray(inputs["y"], dtype=np.float32)[b]
    s = np.asarray(inputs["s"], dtype=np.float32)[b]
    ypepad = np.ascontiguousarray(
        reflect_pad((y + _PE_Y).astype(np.float32)).reshape(2, 128, 66, 66))
    yreppad = np.ascontiguousarray(edge_pad(y).reshape(2, 128, 66, 66))
    spepad = np.ascontiguousarray(reflect_pad((s + _PE_S).astype(np.float32)))
    _BATCH_CACHE.clear()
    _BATCH_CACHE[key] = (ypepad, yreppad, spepad)
    return _BATCH_CACHE[key]


def prepare_core_inputs(inputs, core):
    global _PE_Y, _PE_S
    if _PE_Y is None:
        _PE_Y = pos_encoding_pe(256, 64 * 64).reshape(256, 64, 64)
        _PE_S = pos_encoding_pe(128, 128 * 128).reshape(128, 128, 128)
    b, h = core // 4, core % 4
    ch = slice(32 * h, 32 * h + 32)
    s = np.asarray(inputs["s"], dtype=np.float32)[b]

    ypepad, yreppad, spepad = _batch_shared(inputs, b)
    sgate = arrange_to_strips(np.ascontiguousarray(s[ch]))

    w_blue_y = np.asarray(inputs["w_blue_y"], dtype=np.float32)[ch]
    w_blue_s = np.asarray(inputs["w_blue_s"], dtype=np.float32)[ch]
    w_green = np.asarray(inputs["w_green"], dtype=np.float32)[ch]
    w_purple = np.asarray(inputs["w_purple"], dtype=np.float32)[ch]

    wq = np.zeros((18, 128, 128), dtype=np.float32)
    for t in range(9):
        ky, kx = t // 3, t % 3
        for kt in range(2):
            blk = w_blue_y[:, 128 * kt : 128 * kt + 128, ky, kx].T
            wq[t * 2 + kt] = np.tile(blk, (1, 4))
    wv = np.zeros((9, 128, 32), dtype=np.float32)
    for t in range(9):
        ky, kx = t // 3, t % 3
        wv[t] = w_blue_s[:, :, ky, kx].T

    def make_w9(w):
        # W9[ey, ex][ci, 32*p+c] = W2[p, ey-pr, ex-pc][ci, c] (0 if invalid):
        # all four phases computed from one 9-tap pass over the edge-padded
        # original-resolution image, phase-major on output partitions.
        W2 = collapse_w2(w)                      # [4, 2, 2, ci, co32]
        ci = W2.shape[3]
        W9 = np.zeros((3, 3, ci, 128), dtype=np.float32)
        for p in range(4):
            pr, pc = p // 2, p % 2
            for dy in range(2):
                for dx in range(2):
                    W9[pr + dy, pc + dx, :, 32 * p : 32 * p + 32] = W2[p, dy, dx]
        return W9

    W9g = make_w9(w_green)                       # [3, 3, 256, 128]
    wg = W9g.reshape(3, 3, 2, 128, 128).transpose(0, 1, 2, 3, 4).reshape(18, 128, 128).copy()
    W9p = make_w9(w_purple)                      # [3, 3, 128, 128]
    wp = W9p.reshape(9, 128, 128).copy()

    affg = np.stack(
        [np.asarray(inputs["rho_g"], dtype=np.float32).reshape(128)[ch],
         np.asarray(inputs["gamma_g"], dtype=np.float32).reshape(128)[ch],
         np.asarray(inputs["beta_g"], dtype=np.float32).reshape(128)[ch]],
        axis=1)
    affp = np.stack(
        [np.asarray(inputs["rho_p"], dtype=np.float32).reshape(128)[ch],
         np.asarray(inputs["gamma_p"], dtype=np.float32).reshape(128)[ch],
         np.asarray(inputs["beta_p"], dtype=np.float32).reshape(128)[ch]],
        axis=1)

    sel = np.zeros((128, 32), dtype=np.float32)
    for p in range(128):
        sel[p, p % 32] = 1.0

    return {
        "ypepad": ypepad,
        "yreppad": yreppad,
        "spepad": spepad,
        "sgate": sgate,
        "wq": wq,
        "wv": wv,
        "wg": wg,
        "wp": wp,
        "affg": np.ascontiguousarray(affg),
        "affp": np.ascontiguousarray(affp),
        "sel": sel,
        "onesr": np.ones((128, 1), dtype=np.float32),
    }


def assemble_output(per_core_z, per_core_upy):
    out = np.zeros((2, 256, 128, 128), dtype=np.float32)
    for core in range(8):
        b, h = core // 4, core % 4
        out[b, 32 * h : 32 * h + 32] = unarrange_from_strips(per_core_z[core])
        out[b, 128 + 32 * h : 128 + 32 * h + 32] = unarrange_from_strips(per_core_upy[core])
    return out


# ----------------------------------------------------------------------------
# Bass kernel
# ----------------------------------------------------------------------------

def build_bass(loop_n=None, no_cc=False):
    import concourse.bass as bass
    import concourse.tile as tile
    from concourse import bacc, mybir

    f32 = mybir.dt.float32
    f32r = mybir.dt.float32r
    AF = mybir.ActivationFunctionType
    ALU = mybir.AluOpType

    def r32(ap):
        return ap.bitcast(f32r)

    nc = bacc.Bacc(num_devices=8)

    # ---- I/O ----
    ypepad_d = nc.declare_dram_parameter("ypepad", [2, 128, 66, 66], f32r, isOutput=False)
    yreppad_d = nc.declare_dram_parameter("yreppad", [2, 128, 66, 66], f32r, isOutput=False)
    spepad_d = nc.declare_dram_parameter("spepad", [128, 130, 130], f32r, isOutput=False)
    sgate_d = nc.declare_dram_parameter("sgate", [128, 4096], f32, isOutput=False)
    wq_d = nc.declare_dram_parameter("wq", [18, 128, 128], f32r, isOutput=False)
    wv_d = nc.declare_dram_parameter("wv", [9, 128, 32], f32r, isOutput=False)
    wg_d = nc.declare_dram_parameter("wg", [18, 128, 128], f32r, isOutput=False)
    wp_d = nc.declare_dram_parameter("wp", [9, 128, 128], f32r, isOutput=False)
    affg_d = nc.declare_dram_parameter("affg", [32, 3], f32, isOutput=False)
    affp_d = nc.declare_dram_parameter("affp", [32, 3], f32, isOutput=False)
    sel_d = nc.declare_dram_parameter("sel", [128, 32], f32, isOutput=False)
    onesr_d = nc.declare_dram_parameter("onesr", [128, 1], f32r, isOutput=False)
    zout_d = nc.declare_dram_parameter("zout", [128, 4096], f32, isOutput=True)
    upyout_d = nc.declare_dram_parameter("upyout", [128, 4096], f32, isOutput=True)

    # ---- internal DRAM (collective bounce buffers etc.) ----
    cc1_in = nc.dram_tensor("cc1_in", [32, 4096], f32r)
    cc1_out = nc.dram_tensor("cc1_out", [128, 4096], f32r)
    cc2_in = nc.dram_tensor("cc2_in", [1, 4], f32)
    cc2_out = nc.dram_tensor("cc2_out", [1, 4], f32)
    rsumb = nc.dram_tensor("rsumb", [4096], f32)
    rsumb2 = nc.dram_tensor("rsumb2", [4096], f32)

    import contextlib

    with tile.TileContext(nc) as tc, contextlib.ExitStack() as ctx:
        pers = ctx.enter_context(tc.tile_pool(name="pers", bufs=1))
        small = ctx.enter_context(tc.tile_pool(name="small", bufs=1))

        # ---------------- weights + constants ----------------
        wq_sb = pers.tile([128, 18, 128], f32r, tag="wq")
        nc.sync.dma_start(out=wq_sb, in_=wq_d[:, :, :].rearrange("t p m -> p t m"))
        wv_sb = pers.tile([128, 9, 32], f32r, tag="wv")
        nc.sync.dma_start(out=wv_sb, in_=wv_d[:, :, :].rearrange("t p m -> p t m"))
        wg_sb = pers.tile([128, 18, 128], f32r, tag="wg")
        nc.sync.dma_start(out=wg_sb, in_=wg_d[:, :, :].rearrange("t p m -> p t m"))
        wp_sb = pers.tile([128, 9, 128], f32r, tag="wp")
        nc.sync.dma_start(out=wp_sb, in_=wp_d[:, :, :].rearrange("t p m -> p t m"))
        sel_sb = pers.tile([128, 32], f32, tag="sel")
        nc.sync.dma_start(out=sel_sb, in_=sel_d[:, :])
        ones_sb = pers.tile([128, 1], f32, tag="ones")
        nc.vector.memset(ones_sb, 1.0)
        affg_sb = small.tile([32, 3], f32, tag="affg")
        nc.sync.dma_start(out=affg_sb, in_=affg_d[:, :])
        affp_sb = small.tile([32, 3], f32, tag="affp")
        nc.sync.dma_start(out=affp_sb, in_=affp_d[:, :])

        def rsqrt_col(x, p, tag, eps=EPS):
            """[p, 1] tile -> rsqrt(x + eps), via reciprocal + Sqrt ACT +
            one Newton step (y*(1.5 - 0.5*xe*y^2)) to clean up sqrt ULPs."""
            xe = small.tile([p, 1], f32, tag=tag + "xe", name=tag + "xe")
            nc.vector.tensor_scalar_add(xe, x, eps)
            r = small.tile([p, 1], f32, tag=tag + "r", name=tag + "r")
            nc.vector.reciprocal(out=r, in_=xe)
            y = small.tile([p, 1], f32, tag=tag + "y", name=tag + "y")
            nc.scalar.activation(out=y, in_=r, func=AF.Sqrt)
            t = small.tile([p, 1], f32, tag=tag + "nt", name=tag + "nt")
            nc.vector.tensor_mul(t, y, y)
            nc.vector.tensor_mul(t, t, xe)
            nc.vector.tensor_scalar(out=t, in0=t, scalar1=-0.5, scalar2=1.5,
                                    op0=ALU.mult, op1=ALU.add)
            nc.vector.tensor_mul(y, y, t)
            return y

        def emit_body():
            attn = ctx.enter_context(tc.tile_pool(name="attn", bufs=1))
            # =========== v conv (blue_s, stride 2, M=32) ===========
            vT_sb = attn.tile([128, 33 * 32], f32r, tag="vT")
            vT_ones_view = vT_sb.rearrange("p (jb c) -> p jb c", c=33)[:, :, 32:33]
            nc.sync.dma_start(
                out=vT_ones_view,
                in_=bass.AP(tensor=onesr_d, offset=0, ap=[[1, 128], [0, 32], [0, 1]]))
            vstats = small.tile([32, 8, 6], f32, tag="vstats")
            with tc.tile_pool(name="vsec", bufs=1) as vsec, \
                 tc.tile_pool(name="vtmp", bufs=2) as vtmp_pool, \
                 tc.tile_pool(name="cps2", bufs=3, space="PSUM") as cps2:
                spe = vsec.tile([128, 130, 130], f32r, tag="spe")
                for rb in range(5):
                    r0, r1 = 26 * rb, 26 * rb + 26
                    nc.sync.dma_start(out=spe[:, r0:r1, :], in_=spepad_d[:, r0:r1, :])
                vraw = vtmp_pool.tile([32, 4096], f32, tag="vtmp", name="vraw")
                for chunk in range(8):
                    vps = cps2.tile([128, 512], f32, tag="vps")
                    r0 = 8 * chunk
                    for t in range(9):
                        ky, kx = t // 3, t % 3
                        nc.tensor.matmul(
                            vps[0:32, :],
                            wv_sb[:, t, :],
                            spe[:, 2 * r0 + ky : 2 * r0 + ky + 16 : 2, kx : kx + 128 : 2],
                            start=(t == 0), stop=(t == 8),
                        )
                    nc.vector.tensor_copy(vraw[:, 512 * chunk : 512 * chunk + 512], vps[0:32, :])
                    nc.vector.bn_stats(out=vstats[:, chunk, :], in_=vraw[:, 512 * chunk : 512 * chunk + 512])

                vmv = small.tile([32, 2], f32, tag="vmv")
                nc.vector.bn_aggr(out=vmv, in_=vstats)
                vinv = rsqrt_col(vmv[:, 1:2], 32, "vinv")
                vbias = small.tile([32, 1], f32, tag="vbias")
                nc.vector.tensor_scalar(out=vbias, in0=vmv[:, 0:1], scalar1=vinv, scalar2=-1.0,
                                        op0=ALU.mult, op1=ALU.mult)
                v2d = vtmp_pool.tile([32, 4096], f32, tag="vtmp", name="v2d")
                nc.scalar.activation(out=v2d, in_=vraw, func=AF.Silu, bias=vbias, scale=vinv)

                # vT with ones column: vT_sb[32w+i, 33*jb+c] = v2d[c, 128*jb+32w+i]
                vt32 = vtmp_pool.tile([32, 4096], f32, tag="vtmp", name="vt32")
                nc.vector.transpose(out=vt32, in_=v2d)
                vt32_v = vt32.rearrange("p (m c) -> p m c", c=32)   # m = 4*jb + w
                vT_v = vT_sb.rearrange("p (jb c) -> p jb c", c=33)
                for w in range(4):
                    nc.gpsimd.dma_start(
                        out=vT_v[32 * w : 32 * w + 32, :, 0:32],
                        in_=vt32_v[:, w::4, :])

            # =========== q conv (blue_y, M=128 replicated) ===========
            qstats = small.tile([128, 8, 6], f32, tag="qstats")
            with tc.tile_pool(name="qsec", bufs=1) as qsec, \
                 tc.tile_pool(name="cps1", bufs=3, space="PSUM") as cps1:
                ype = [qsec.tile([128, 66, 66], f32r, tag=f"ype{kt}", name=f"ype{kt}") for kt in range(2)]
                for kt in range(2):
                    for rb in range(3):
                        r0, r1 = 22 * rb, 22 * rb + 22
                        nc.sync.dma_start(out=ype[kt][:, r0:r1, :], in_=ypepad_d[kt][:, r0:r1, :])
                qraw = qsec.tile([128, 4096], f32, tag="qraw")
                for chunk in range(8):
                    qps = cps1.tile([128, 512], f32, tag="qps")
                    r0 = 8 * chunk
                    idx = 0
                    for t in range(9):
                        ky, kx = t // 3, t % 3
                        for kt in range(2):
                            nc.tensor.matmul(
                                qps[:, :],
                                wq_sb[:, t * 2 + kt, :],
                                ype[kt][:, r0 + ky : r0 + ky + 8, kx : kx + 64],
                                start=(idx == 0), stop=(idx == 17),
                            )
                            idx += 1
                    nc.vector.tensor_copy(qraw[:, 512 * chunk : 512 * chunk + 512], qps[:, :])
                    nc.vector.bn_stats(out=qstats[:, chunk, :], in_=qraw[:, 512 * chunk : 512 * chunk + 512])

                qmv = small.tile([128, 2], f32, tag="qmv")
                nc.vector.bn_aggr(out=qmv, in_=qstats)
                qinv = rsqrt_col(qmv[:, 1:2], 128, "qinv")
                qbias = small.tile([128, 1], f32, tag="qbias")
                nc.vector.tensor_scalar(out=qbias, in0=qmv[:, 0:1], scalar1=qinv, scalar2=-1.0,
                                        op0=ALU.mult, op1=ALU.mult)
                qrep = attn.tile([128, 4096], f32r, tag="qrep")
                nc.scalar.activation(out=qrep, in_=qraw, func=AF.Silu, bias=qbias, scale=qinv)

            # =========== attention + interleaved green conv ===========
            greenraw = ctx.enter_context(tc.tile_pool(name="gpool", bufs=1)).tile(
                [128, 4096], f32, tag="greenraw", name="greenraw")
            gstats = small.tile([128, 8, 6], f32, tag="gstats")
            mharaw = attn.tile([33, 4096], f32, tag="mharaw")

            with tc.tile_pool(name="yrep", bufs=1) as yrep_pool, \
                 tc.tile_pool(name="aexpp", bufs=4) as aexp_pool, \
                 tc.tile_pool(name="gps", bufs=2, space="PSUM") as gps_pool, \
                 tc.tile_pool(name="qkps", bufs=2, space="PSUM") as qkps, \
                 tc.tile_pool(name="pvps", bufs=2, space="PSUM") as pvps:
                yrep = [yrep_pool.tile([128, 66, 66], f32r, tag=f"yrep{kt}", name=f"yrep{kt}") for kt in range(2)]
                for kt in range(2):
                    for rb in range(3):
                        r0, r1 = 22 * rb, 22 * rb + 22
                        nc.sync.dma_start(out=yrep[kt][:, r0:r1, :], in_=yreppad_d[kt][:, r0:r1, :])

                green_tiles = {}

                def green_piece(piece):
                    # piece = (chunk, sub) with sub in 0..5 -> 3 MMs each
                    chunk, sub = piece // 6, piece % 6
                    r0 = 8 * chunk
                    if sub == 0:
                        green_tiles[chunk] = gps_pool.tile(
                            [128, 512], f32, tag="gpsum", name=f"g{chunk}")
                    gtile = green_tiles[chunk]
                    for k in range(3):
                        idx = sub * 3 + k
                        tap, kt = idx // 2, idx % 2
                        ey, ex = tap // 3, tap % 3
                        nc.tensor.matmul(
                            gtile[:, :],
                            wg_sb[:, tap * 2 + kt, :],
                            yrep[kt][:, r0 + ey : r0 + ey + 8, ex : ex + 64],
                            start=(idx == 0), stop=(idx == 17),
                        )
                    if sub == 5:
                        col = 512 * chunk
                        nc.vector.tensor_copy(greenraw[:, col : col + 512], gtile[:, :])
                        nc.vector.bn_stats(out=gstats[:, chunk, :], in_=greenraw[:, col : col + 512])
                        del green_tiles[chunk]

                vT_v = vT_sb.rearrange("p (jb c) -> p jb c", c=33)
                gu_next = 0
                it = 0
                for I in range(8):
                    pvt = pvps.tile([128, 512], f32, tag="pvt", name=f"pvt{I}")
                    for g in range(16):
                        qk = qkps.tile([128, 1024], f32, tag="qk", name=f"qk{I}_{g}")
                        for t in range(2):
                            jb = 2 * g + t
                            nc.tensor.matmul(
                                qk[:, 512 * t : 512 * t + 512],
                                qrep[0:32, 128 * jb : 128 * jb + 128],
                                qrep[0:32, 512 * I : 512 * I + 512],
                                start=True, stop=True,
                            )
                        aexp = aexp_pool.tile([128, 1024], f32r, tag="aexp", name=f"ae{I}_{g}")
                        nc.scalar.activation(out=aexp, in_=qk, func=AF.Exp, scale=SCALE)
                        for t in range(2):
                            jb = 2 * g + t
                            nc.tensor.matmul(
                                pvt[0:33, :],
                                vT_v[:, jb, :],
                                aexp[:, 512 * t : 512 * t + 512],
                                start=(g == 0 and t == 0), stop=(g == 15 and t == 1),
                                skip_group_check=True,
                            )
                        it += 1
                        if it % 2 == 0 and gu_next < 48:
                            green_piece(gu_next)
                            gu_next += 1
                    nc.vector.tensor_copy(mharaw[:, 512 * I : 512 * I + 512], pvt[0:33, :])

            gmv = small.tile([128, 2], f32, tag="gmv")
            nc.vector.bn_aggr(out=gmv, in_=gstats)

            # =========== softmax denominators + divide ===========
            with tc.tile_pool(name="divp", bufs=1) as divp:
                nc.sync.dma_start(out=bass.AP(tensor=rsumb, offset=0, ap=[[1, 4096]]),
                                  in_=mharaw[32:33, :])
                rsq = small.tile([128, 32], f32, tag="rsq")
                nc.sync.dma_start(out=rsq, in_=bass.AP(tensor=rsumb, offset=0, ap=[[32, 128], [1, 32]]))
                nc.vector.reciprocal(out=rsq, in_=rsq)
                nc.sync.dma_start(out=bass.AP(tensor=rsumb2, offset=0, ap=[[32, 128], [1, 32]]), in_=rsq)
                rs32 = divp.tile([32, 4096], f32, tag="rs32")
                nc.sync.dma_start(out=rs32,
                                  in_=bass.AP(tensor=rsumb2, offset=0, ap=[[0, 32], [1, 4096]]))
                mha2db = divp.tile([32, 4096], f32r, tag="mha2db")
                nc.vector.tensor_mul(mha2db, mharaw[0:32, :], rs32)

                # AllGather mha across the 4 cores of this batch
                nc.sync.dma_start(out=cc1_in[:, :], in_=mha2db)
                if no_cc:
                    for g in range(4):
                        nc.sync.dma_start(out=cc1_out[32 * g : 32 * g + 32, :], in_=cc1_in[:, :])
                else:
                    nc.gpsimd.collective_compute(
                        "AllGather", mybir.AluOpType.bypass,
                        replica_groups=_REPLICA_GROUPS,
                        ins=[cc1_in[:, :]],
                        outs=[cc1_out[:, :]],
                    )

            # =========== purple conv ===========
            with tc.tile_pool(name="tailp", bufs=1) as tailp, \
                 tc.tile_pool(name="gps2", bufs=2, space="PSUM") as gps2, \
                 tc.tile_pool(name="tailps", bufs=2, space="PSUM") as tailps:
                mhapad = tailp.tile([128, 66, 66], f32r, tag="mhapad")
                cc1_v = cc1_out.rearrange("p (r c) -> p r c", c=64)
                nc.sync.dma_start(out=mhapad[:, 1:65, 1:65], in_=cc1_v)
                nc.sync.dma_start(out=mhapad[:, 0:1, 1:65], in_=cc1_v[:, 0:1, :])
                nc.sync.dma_start(out=mhapad[:, 65:66, 1:65], in_=cc1_v[:, 63:64, :])
                nc.sync.dma_start(out=mhapad[:, 0:66, 0:1], in_=mhapad[:, 0:66, 1:2])
                nc.sync.dma_start(out=mhapad[:, 0:66, 65:66], in_=mhapad[:, 0:66, 64:65])

                purpleraw = tailp.tile([128, 4096], f32, tag="purpleraw")
                pstats = small.tile([128, 8, 6], f32, tag="pstats")
                for chunk in range(8):
                    ptile = gps2.tile([128, 512], f32, tag="gpsum2", name=f"pt{chunk}")
                    r0 = 8 * chunk
                    for tap in range(9):
                        ey, ex = tap // 3, tap % 3
                        nc.tensor.matmul(
                            ptile[:, :],
                            wp_sb[:, tap, :],
                            mhapad[:, r0 + ey : r0 + ey + 8, ex : ex + 64],
                            start=(tap == 0), stop=(tap == 8),
                        )
                    col = 512 * chunk
                    nc.vector.tensor_copy(purpleraw[:, col : col + 512], ptile[:, :])
                    nc.vector.bn_stats(out=pstats[:, chunk, :], in_=purpleraw[:, col : col + 512])
                pmv = small.tile([128, 2], f32, tag="pmv")
                nc.vector.bn_aggr(out=pmv, in_=pstats)

                # ---- stats -> sums, channel combine, allreduce ----
                def part_sums(mv, tag):
                    s2 = small.tile([128, 2], f32, tag=tag, name=tag)
                    nc.vector.tensor_scalar_mul(s2[:, 0:1], mv[:, 0:1], 4096.0)
                    t = small.tile([128, 1], f32, tag=tag + "t", name=tag + "t")
                    nc.vector.tensor_mul(t, mv[:, 0:1], mv[:, 0:1])
                    nc.vector.tensor_add(t, t, mv[:, 1:2])
                    nc.vector.tensor_scalar_mul(s2[:, 1:2], t, 4096.0)
                    return s2

                gsums2 = part_sums(gmv, "gsums2")
                psums2 = part_sums(pmv, "psums2")

                chps = tailps.tile([128, 512], f32, tag="tps", name="chps")
                nc.tensor.matmul(chps[0:32, 0:2], sel_sb, gsums2, start=True, stop=True)
                gch = small.tile([32, 2], f32, tag="gch")
                nc.vector.tensor_copy(gch, chps[0:32, 0:2])
                chps2 = tailps.tile([128, 512], f32, tag="tps", name="chps2")
                nc.tensor.matmul(chps2[0:32, 0:2], sel_sb, psums2, start=True, stop=True)
                pch = small.tile([32, 2], f32, tag="pch")
                nc.vector.tensor_copy(pch, chps2[0:32, 0:2])

                lps = tailps.tile([128, 512], f32, tag="tps", name="lps")
                nc.tensor.matmul(lps[0:1, 0:2], ones_sb, gsums2, start=True, stop=True)
                nc.tensor.matmul(lps[0:1, 2:4], ones_sb, psums2, start=True, stop=True)
                lsb = small.tile([1, 4], f32, tag="lsb")
                nc.vector.tensor_copy(lsb, lps[0:1, 0:4])
                nc.sync.dma_start(out=cc2_in[:, :], in_=lsb)
                if no_cc:
                    nc.sync.dma_start(out=cc2_out[:, :], in_=cc2_in[:, :])
                else:
                    nc.gpsimd.collective_compute(
                        "AllReduce", mybir.AluOpType.add,
                        replica_groups=_REPLICA_GROUPS,
                        ins=[cc2_in[:, :]],
                        outs=[cc2_out[:, :]],
                    )
                lng = small.tile([32, 4], f32, tag="lng")
                nc.sync.dma_start(out=lng, in_=bass.AP(tensor=cc2_out, offset=0, ap=[[0, 32], [1, 4]]))

                # ---- ILN affines ----
                def iln_affine(ch_sums, S_col, aff_sb, tag):
                    n, n1 = N_PX, N_PX - 1.0
                    nt, nt1 = N_TOT, N_TOT - 1.0
                    in_m = small.tile([32, 1], f32, tag=tag + "im", name=tag + "im")
                    nc.vector.tensor_scalar_mul(in_m, ch_sums[:, 0:1], 1.0 / n)
                    t1 = small.tile([32, 1], f32, tag=tag + "t1", name=tag + "t1")
                    nc.vector.tensor_mul(t1, ch_sums[:, 0:1], ch_sums[:, 0:1])
                    nc.vector.tensor_scalar_mul(t1, t1, 1.0 / n)
                    nc.vector.tensor_sub(t1, ch_sums[:, 1:2], t1)
                    in_v = small.tile([32, 1], f32, tag=tag + "iv", name=tag + "iv")
                    nc.vector.tensor_scalar_mul(in_v, t1, 1.0 / n1)
                    inv_in = rsqrt_col(in_v, 32, tag + "ii")

                    ln_m = small.tile([32, 1], f32, tag=tag + "lm", name=tag + "lm")
                    nc.vector.tensor_scalar_mul(ln_m, S_col[:, 0:1], 1.0 / nt)
                    l1 = small.tile([32, 1], f32, tag=tag + "l1", name=tag + "l1")
                    nc.vector.tensor_mul(l1, S_col[:, 0:1], S_col[:, 0:1])
                    nc.vector.tensor_scalar_mul(l1, l1, 1.0 / nt)
                    nc.vector.tensor_sub(l1, S_col[:, 1:2], l1)
                    ln_v = small.tile([32, 1], f32, tag=tag + "lv", name=tag + "lv")
                    nc.vector.tensor_scalar_mul(ln_v, l1, 1.0 / nt1)
                    inv_ln = rsqrt_col(ln_v, 32, tag + "il")

                    rho = aff_sb[:, 0:1]
                    t3 = small.tile([32, 1], f32, tag=tag + "t3", name=tag + "t3")
                    nc.vector.tensor_mul(t3, rho, inv_in)
                    t6 = small.tile([32, 1], f32, tag=tag + "t6", name=tag + "t6")
                    nc.vector.tensor_mul(t6, rho, inv_ln)
                    nc.vector.tensor_sub(t6, inv_ln, t6)
                    A = small.tile([32, 1], f32, tag=tag + "A", name=tag + "A")
                    nc.vector.tensor_add(A, t3, t6)
                    u1 = small.tile([32, 1], f32, tag=tag + "u1", name=tag + "u1")
                    nc.vector.tensor_mul(u1, in_m, t3)
                    u2 = small.tile([32, 1], f32, tag=tag + "u2", name=tag + "u2")
                    nc.vector.tensor_mul(u2, ln_m, t6)
                    nc.vector.tensor_add(u1, u1, u2)
                    B = small.tile([32, 1], f32, tag=tag + "B", name=tag + "B")
                    nc.vector.tensor_scalar_mul(B, u1, -1.0)
                    sb = small.tile([32, 2], f32, tag=tag + "sb", name=tag + "sb")
                    nc.vector.tensor_mul(sb[:, 0:1], A, aff_sb[:, 1:2])
                    nc.vector.tensor_mul(sb[:, 1:2], B, aff_sb[:, 1:2])
                    nc.vector.tensor_add(sb[:, 1:2], sb[:, 1:2], aff_sb[:, 2:3])
                    return sb

                gsb = iln_affine(gch, lng[:, 0:2], affg_sb, "ga")
                psb = iln_affine(pch, lng[:, 2:4], affp_sb, "pa")

                gsb128 = small.tile([128, 2], f32, tag="gsb128")
                psb128 = small.tile([128, 2], f32, tag="psb128")
                nc.sync.dma_start(out=gsb128[0:32, :], in_=gsb)
                nc.sync.dma_start(out=psb128[0:32, :], in_=psb)
                for o in (32, 64, 96):
                    nc.sync.dma_start(out=gsb128[o : o + 32, :], in_=gsb128[0:32, :])
                    nc.sync.dma_start(out=psb128[o : o + 32, :], in_=psb128[0:32, :])

                # ---- finalize outputs ----
                sgate_sb = tailp.tile([128, 4096], f32, tag="sgate")
                nc.sync.dma_start(out=sgate_sb, in_=sgate_d[:, :])

                upy_sb = tailp.tile([128, 4096], f32, tag="upy")
                nc.scalar.activation(out=upy_sb, in_=greenraw, func=AF.Silu,
                                     bias=gsb128[:, 1:2], scale=gsb128[:, 0:1])
                nc.sync.dma_start(out=upyout_d[:, :], in_=upy_sb)

                zpre = tailp.tile([128, 4096], f32, tag="zpre")
                nc.scalar.activation(out=zpre, in_=purpleraw, func=AF.Sigmoid,
                                     bias=psb128[:, 1:2], scale=psb128[:, 0:1])
                nc.vector.tensor_mul(zpre, zpre, sgate_sb)
                nc.sync.dma_start(out=zout_d[:, :], in_=zpre)

        if loop_n is None:
            emit_body()
        else:
            with tc.For_i(0, loop_n, 1):
                emit_body()

    nc.compile()
    return nc


_NC_CACHE = None
RUN_KWARGS = {}      # test harness may set e.g. {"trace": True}
LAST_RESULTS = None  # BassKernelResults of the most recent run


def kernel(**inputs) -> np.ndarray:
    global _NC_CACHE, LAST_RESULTS
    from concourse.bass_utils import run_bass_kernel_spmd

    if _NC_CACHE is None:
        _NC_CACHE = build_bass()
    nc = _NC_CACHE

    in_maps = []
    for core in _CORES:
        ci = prepare_core_inputs(inputs, core)
        in_maps.append(ci)

    res = run_bass_kernel_spmd(nc, in_maps, _CORES, **RUN_KWARGS)
    LAST_RESULTS = res
    zs = [res.results[c]["zout"] for c in _CORES]
    upys = [res.results[c]["upyout"] for c in _CORES]
    return assemble_output(zs, upys)


if __name__ == "__main__":
    nc = build_bass()
    print("bass build OK")



# revision 5
# speedup vs baseline: 1.0720x; 1.0720x over previous
"""Trainium2 Bass kernel for nn_MultiHeadCrossAttention_47519518163418.

Sharding: 8 cores = (batch b in {0,1}) x (head h in {0..3}); core c: b=c//4, h=c%4.
Each core computes q/k/v for its head's 32 channels, runs the full 4096x4096
attention for that head (scores transposed, softmax without max subtraction,
row sums via an appended ones-column in the PV matmul), then the 4 cores of a
batch AllGather the attention output (bf16, column-pre-padded) to form the
128-channel conv input for the purple path. Green and purple paths use a
phase-collapsed 2x2-tap decomposition of "upsample2 + reflect-pad + 3x3 conv".
Green conv is emitted after the AllGather launch so its PE work hides the
collective; purple handles row clamping with split matmuls so no row-padded
copy is needed. All large inputs are bf16 with contiguous per-partition DMA
layouts. ILN layer statistics are combined with two tiny AllReduces.
"""

import sys

if "/opt/trn_rl_repo" not in sys.path:
    sys.path.insert(0, "/opt/trn_rl_repo")

import numpy as np
import ml_dtypes

BF16 = ml_dtypes.bfloat16

NUM_HEADS = 4
EPS = 1e-5
D_HEAD = 32
SCALE = float(D_HEAD) ** -0.5
N_PX = 16384.0          # pixels per channel of the upsampled image
N_TOT = 128 * 16384.0   # elements per batch for layer stats

_CORES = list(range(8))
_REPLICA_GROUPS = [[0, 1, 2, 3], [4, 5, 6, 7]]


# ----------------------------------------------------------------------------
# Host-side helpers
# ----------------------------------------------------------------------------

def pos_encoding_pe(c, L, dtype=np.float32):
    half = c // 2
    pos = np.arange(L, dtype=dtype)
    depths = np.arange(half, dtype=dtype) / half
    rates = 1.0 / (10000.0 ** depths)
    ang = pos[:, None] * rates[None, :]
    pe = np.concatenate([np.sin(ang), np.cos(ang)], axis=-1)  # [L, c]
    return pe.T.astype(dtype)  # [c, L]


def reflect_pad(x):
    return np.pad(x, ((0, 0), (1, 1), (1, 1)), mode="reflect")


def edge_pad(x):
    return np.pad(x, ((0, 0), (1, 1), (1, 1)), mode="edge")


_KSET = {(0, 0): [0], (0, 1): [1, 2], (1, 0): [0, 1], (1, 1): [2]}


def collapse_w2(w):
    """w [co, ci, 3, 3] -> W2 [4 (p=2*pr+pc), 2 (dy), 2 (dx), ci, co]."""
    co, ci = w.shape[0], w.shape[1]
    W2 = np.zeros((4, 2, 2, ci, co), dtype=w.dtype)
    for pr in range(2):
        for pc in range(2):
            p = 2 * pr + pc
            for dy in range(2):
                for dx in range(2):
                    acc = np.zeros((co, ci), dtype=np.float64)
                    for ky in _KSET[(pr, dy)]:
                        for kx in _KSET[(pc, dx)]:
                            acc = acc + w[:, :, ky, kx].astype(np.float64)
                    W2[p, dy, dx] = acc.T.astype(w.dtype)
    return W2


def arrange_to_strips(x2d):
    """x [32, 128, 128] -> arranged [128, 4096] phase-major: partition
    32*(2*pr+pc)+c, free r*64+cc for upsampled pixel (2r+pr, 2cc+pc)."""
    t = x2d.reshape(32, 64, 2, 64, 2)          # c, r, pr, cc, pc
    t = t.transpose(2, 4, 0, 1, 3)              # pr, pc, c, r, cc
    return np.ascontiguousarray(t.reshape(128, 4096))


def unarrange_from_strips(arr):
    t = arr.reshape(2, 2, 32, 64, 64)           # pr, pc, c, r, cc
    t = t.transpose(2, 3, 0, 4, 1)              # c, r, pr, cc, pc
    return np.ascontiguousarray(t.reshape(32, 128, 128))


_PE_Y = None
_PE_S = None

_BATCH_CACHE = {}


def _batch_shared(inputs, b):
    """Padded/PE-added tensors shared by the 4 cores of a batch (bf16)."""
    key = (id(inputs), b)
    if key in _BATCH_CACHE:
        return _BATCH_CACHE[key]
    y = np.asarray(inputs["y"], dtype=np.float32)[b]
    s = np.asarray(inputs["s"], dtype=np.float32)[b]
    ypepad = reflect_pad((y + _PE_Y).astype(np.float32))       # [256, 66, 66]
    ypepad = np.ascontiguousarray(
        ypepad.reshape(2, 128, 66 * 66)).astype(BF16)
    yreppad = np.ascontiguousarray(
        edge_pad(y).reshape(2, 128, 66 * 66)).astype(BF16)
    spepad = reflect_pad((s + _PE_S).astype(np.float32))       # [128, 130, 130]
    # even/odd column split: spe_eo[p, r, e, c] = spepad[p, r, 2c+e]
    spe_eo = spepad.reshape(128, 130, 65, 2).transpose(0, 1, 3, 2)
    spe_eo = np.ascontiguousarray(spe_eo.reshape(128, 130 * 130)).astype(BF16)
    _BATCH_CACHE.clear()
    _BATCH_CACHE[key] = (ypepad, yreppad, spe_eo)
    return _BATCH_CACHE[key]


def prepare_core_inputs(inputs, core):
    global _PE_Y, _PE_S
    if _PE_Y is None:
        _PE_Y = pos_encoding_pe(256, 64 * 64).reshape(256, 64, 64)
        _PE_S = pos_encoding_pe(128, 128 * 128).reshape(128, 128, 128)
    b, h = core // 4, core % 4
    ch = slice(32 * h, 32 * h + 32)
    s = np.asarray(inputs["s"], dtype=np.float32)[b]

    ypepad, yreppad, spe_eo = _batch_shared(inputs, b)
    sgate = arrange_to_strips(np.ascontiguousarray(s[ch]))

    w_blue_y = np.asarray(inputs["w_blue_y"], dtype=np.float32)[ch]
    w_blue_s = np.asarray(inputs["w_blue_s"], dtype=np.float32)[ch]
    w_green = np.asarray(inputs["w_green"], dtype=np.float32)[ch]
    w_purple = np.asarray(inputs["w_purple"], dtype=np.float32)[ch]

    wq = np.zeros((18, 128, 128), dtype=np.float32)
    for t in range(9):
        ky, kx = t // 3, t % 3
        for kt in range(2):
            blk = w_blue_y[:, 128 * kt : 128 * kt + 128, ky, kx].T
            wq[t * 2 + kt] = np.tile(blk, (1, 4))
    wv = np.zeros((9, 128, 32), dtype=np.float32)
    for t in range(9):
        ky, kx = t // 3, t % 3
        wv[t] = w_blue_s[:, :, ky, kx].T

    def make_w9(w):
        W2 = collapse_w2(w)                      # [4, 2, 2, ci, co32]
        ci = W2.shape[3]
        W9 = np.zeros((3, 3, ci, 128), dtype=np.float32)
        for p in range(4):
            pr, pc = p // 2, p % 2
            for dy in range(2):
                for dx in range(2):
                    W9[pr + dy, pc + dx, :, 32 * p : 32 * p + 32] = W2[p, dy, dx]
        return W9

    W9g = make_w9(w_green)                       # [3, 3, 256, 128]
    wg = W9g.reshape(3, 3, 2, 128, 128).reshape(18, 128, 128)
    W9p = make_w9(w_purple)                      # [3, 3, 128, 128]
    wp = W9p.reshape(9, 128, 128)

    # flat partition-major weight layouts [ci, t*M + m]
    wq_f = np.ascontiguousarray(wq.transpose(1, 0, 2).reshape(128, 18 * 128)).astype(BF16)
    wv_f = np.ascontiguousarray(wv.transpose(1, 0, 2).reshape(128, 9 * 32)).astype(BF16)
    wg_f = np.ascontiguousarray(wg.transpose(1, 0, 2).reshape(128, 18 * 128)).astype(BF16)
    wp_f = np.ascontiguousarray(wp.transpose(1, 0, 2).reshape(128, 9 * 128)).astype(BF16)

    affg = np.stack(
        [np.asarray(inputs["rho_g"], dtype=np.float32).reshape(128)[ch],
         np.asarray(inputs["gamma_g"], dtype=np.float32).reshape(128)[ch],
         np.asarray(inputs["beta_g"], dtype=np.float32).reshape(128)[ch]],
        axis=1)
    affp = np.stack(
        [np.asarray(inputs["rho_p"], dtype=np.float32).reshape(128)[ch],
         np.asarray(inputs["gamma_p"], dtype=np.float32).reshape(128)[ch],
         np.asarray(inputs["beta_p"], dtype=np.float32).reshape(128)[ch]],
        axis=1)

    sel = np.zeros((128, 32), dtype=np.float32)
    for p in range(128):
        sel[p, p % 32] = 1.0
    selT = np.ascontiguousarray(sel.T)

    return {
        "ypepad": ypepad,
        "yreppad": yreppad,
        "spe": spe_eo,
        "sgate": sgate,
        "wq": wq_f,
        "wv": wv_f,
        "wg": wg_f,
        "wp": wp_f,
        "affg": np.ascontiguousarray(affg),
        "affp": np.ascontiguousarray(affp),
        "sel": sel,
        "selT": selT,
    }


def assemble_output(per_core_z, per_core_upy):
    out = np.zeros((2, 256, 128, 128), dtype=np.float32)
    for core in range(8):
        b, h = core // 4, core % 4
        out[b, 32 * h : 32 * h + 32] = unarrange_from_strips(per_core_z[core])
        out[b, 128 + 32 * h : 128 + 32 * h + 32] = unarrange_from_strips(per_core_upy[core])
    return out


# ----------------------------------------------------------------------------
# Bass kernel
# ----------------------------------------------------------------------------

def build_bass(no_cc=False):
    import concourse.bass as bass
    import concourse.tile as tile
    from concourse import bacc, mybir

    f32 = mybir.dt.float32
    f32r = mybir.dt.float32r
    bf16 = mybir.dt.bfloat16
    AF = mybir.ActivationFunctionType
    ALU = mybir.AluOpType

    nc = bacc.Bacc(num_devices=8)

    # ---- I/O ----
    ype_d = nc.declare_dram_parameter("ypepad", [2, 128, 4356], bf16, isOutput=False)
    yrep_d = nc.declare_dram_parameter("yreppad", [2, 128, 4356], bf16, isOutput=False)
    spe_d = nc.declare_dram_parameter("spe", [128, 16900], bf16, isOutput=False)
    sgate_d = nc.declare_dram_parameter("sgate", [128, 4096], f32, isOutput=False)
    wq_d = nc.declare_dram_parameter("wq", [128, 18 * 128], bf16, isOutput=False)
    wv_d = nc.declare_dram_parameter("wv", [128, 9 * 32], bf16, isOutput=False)
    wg_d = nc.declare_dram_parameter("wg", [128, 18 * 128], bf16, isOutput=False)
    wp_d = nc.declare_dram_parameter("wp", [128, 9 * 128], bf16, isOutput=False)
    affg_d = nc.declare_dram_parameter("affg", [32, 3], f32, isOutput=False)
    affp_d = nc.declare_dram_parameter("affp", [32, 3], f32, isOutput=False)
    sel_d = nc.declare_dram_parameter("sel", [128, 32], f32, isOutput=False)
    selT_d = nc.declare_dram_parameter("selT", [32, 128], f32, isOutput=False)
    zout_d = nc.declare_dram_parameter("zout", [128, 4096], f32, isOutput=True)
    upyout_d = nc.declare_dram_parameter("upyout", [128, 4096], f32, isOutput=True)

    # ---- internal DRAM (collective bounce buffers) ----
    cc1_in = nc.dram_tensor("cc1_in", [32, 4224], bf16)    # 64 rows x 66 padded cols
    cc1_out = nc.dram_tensor("cc1_out", [128, 4224], bf16)
    cc2g_in = nc.dram_tensor("cc2g_in", [1, 2], f32)
    cc2g_out = nc.dram_tensor("cc2g_out", [1, 2], f32)
    cc2p_in = nc.dram_tensor("cc2p_in", [1, 2], f32)
    cc2p_out = nc.dram_tensor("cc2p_out", [1, 2], f32)

    import contextlib

    with tile.TileContext(nc) as tc, contextlib.ExitStack() as ctx:
        pers = ctx.enter_context(tc.tile_pool(name="pers", bufs=1))
        small = ctx.enter_context(tc.tile_pool(name="small", bufs=1))

        # ---------------- weights + constants ----------------
        wq_sb = pers.tile([128, 18 * 128], bf16, tag="wq")
        nc.sync.dma_start(out=wq_sb, in_=wq_d[:, :])
        wq_v = wq_sb.rearrange("p (t m) -> p t m", m=128)
        wv_sb = pers.tile([128, 9 * 32], bf16, tag="wv")
        nc.sync.dma_start(out=wv_sb, in_=wv_d[:, :])
        wv_v = wv_sb.rearrange("p (t m) -> p t m", m=32)
        wg_sb = pers.tile([128, 18 * 128], bf16, tag="wg")
        nc.sync.dma_start(out=wg_sb, in_=wg_d[:, :])
        wg_v = wg_sb.rearrange("p (t m) -> p t m", m=128)
        wp_sb = pers.tile([128, 9 * 128], bf16, tag="wp")
        nc.sync.dma_start(out=wp_sb, in_=wp_d[:, :])
        wp_v = wp_sb.rearrange("p (t m) -> p t m", m=128)
        sel_sb = pers.tile([128, 32], f32, tag="sel")
        nc.sync.dma_start(out=sel_sb, in_=sel_d[:, :])
        selT_sb = pers.tile([32, 128], f32, tag="selT")
        nc.sync.dma_start(out=selT_sb, in_=selT_d[:, :])
        affg_sb = small.tile([32, 3], f32, tag="affg")
        nc.sync.dma_start(out=affg_sb, in_=affg_d[:, :])
        affp_sb = small.tile([32, 3], f32, tag="affp")
        nc.sync.dma_start(out=affp_sb, in_=affp_d[:, :])
        ones_sb = pers.tile([128, 1], f32, tag="ones")
        nc.vector.memset(ones_sb, 1.0)
        ones1 = pers.tile([1, 32], f32, tag="ones1")
        nc.vector.memset(ones1, 1.0)

        def rsqrt_col(x, p, tag, eps=EPS):
            """[p, 1] tile -> rsqrt(x + eps) via reciprocal + Sqrt ACT +
            one Newton step."""
            xe = small.tile([p, 1], f32, tag=tag + "xe", name=tag + "xe")
            nc.vector.tensor_scalar_add(xe, x, eps)
            r = small.tile([p, 1], f32, tag=tag + "r", name=tag + "r")
            nc.vector.reciprocal(out=r, in_=xe)
            y = small.tile([p, 1], f32, tag=tag + "y", name=tag + "y")
            nc.scalar.activation(out=y, in_=r, func=AF.Sqrt)
            t = small.tile([p, 1], f32, tag=tag + "nt", name=tag + "nt")
            nc.vector.tensor_mul(t, y, y)
            nc.vector.tensor_mul(t, t, xe)
            nc.vector.tensor_scalar(out=t, in0=t, scalar1=-0.5, scalar2=1.5,
                                    op0=ALU.mult, op1=ALU.add)
            nc.vector.tensor_mul(y, y, t)
            return y

        gin = ctx.enter_context(tc.tile_pool(name="gin", bufs=1))
        attn = ctx.enter_context(tc.tile_pool(name="attn", bufs=1))

        vT_sb = attn.tile([128, 33 * 32], bf16, tag="vT")
        nc.vector.memset(vT_sb, 1.0)   # ones column survives in col 32 of each block
        vT_v = vT_sb.rearrange("p (jb c) -> p jb c", c=33)

        # =========== v conv (blue_s, stride 2, M=32) ===========
        vstats = small.tile([32, 8, 6], f32, tag="vstats")
        with tc.tile_pool(name="vsec", bufs=1) as vsec, \
             tc.tile_pool(name="cps2", bufs=3, space="PSUM") as cps2:
            spe = vsec.tile([128, 16900], bf16, tag="spe")
            for rb in range(5):
                c0, c1 = 3380 * rb, 3380 * rb + 3380
                nc.sync.dma_start(out=spe[:, c0:c1], in_=spe_d[:, c0:c1])
            spe_v = spe.rearrange("p (r e c) -> p r e c", e=2, c=65)
            vraw = vsec.tile([32, 4096], f32, tag="vraw")
            _EO = {0: (0, 0), 1: (1, 0), 2: (0, 1)}
            for chunk in range(8):
                vps = cps2.tile([128, 512], f32, tag="vps", name=f"vps{chunk}")
                r0 = 8 * chunk
                for t in range(9):
                    ky, kx = t // 3, t % 3
                    e, c0 = _EO[kx]
                    nc.tensor.matmul(
                        vps[0:32, :],
                        wv_v[:, t, :],
                        spe_v[:, 2 * r0 + ky : 2 * r0 + ky + 16 : 2, e, c0 : c0 + 64],
                        start=(t == 0), stop=(t == 8),
                    )
                nc.vector.tensor_copy(vraw[:, 512 * chunk : 512 * chunk + 512], vps[0:32, :])
                nc.vector.bn_stats(out=vstats[:, chunk, :], in_=vraw[:, 512 * chunk : 512 * chunk + 512])

            vmv = small.tile([32, 2], f32, tag="vmv")
            nc.vector.bn_aggr(out=vmv, in_=vstats)
            vinv = rsqrt_col(vmv[:, 1:2], 32, "vinv")
            vbias = small.tile([32, 1], f32, tag="vbias")
            nc.vector.tensor_scalar(out=vbias, in0=vmv[:, 0:1], scalar1=vinv, scalar2=-1.0,
                                    op0=ALU.mult, op1=ALU.mult)
            v2d = vsec.tile([32, 4096], bf16, tag="v2d")
            nc.scalar.activation(out=v2d, in_=vraw, func=AF.Silu, bias=vbias, scale=vinv)

            # vT[32w+i, jb, c] = v2d[c, 128*jb+32w+i]
            vt32 = vsec.tile([32, 4096], bf16, tag="vt32")
            nc.vector.transpose(out=vt32, in_=v2d)
            vt32_v = vt32.rearrange("p (m c) -> p m c", c=32)   # m = 4*jb + w
            for w in range(4):
                nc.gpsimd.dma_start(
                    out=vT_v[32 * w : 32 * w + 32, :, 0:32],
                    in_=vt32_v[:, w::4, :])

        # =========== q conv (blue_y, M=128 replicated) ===========
        qrep = attn.tile([128, 4096], f32r, tag="qrep")
        qstats = small.tile([128, 8, 6], f32, tag="qstats")
        with tc.tile_pool(name="qsec", bufs=1) as qsec, \
             tc.tile_pool(name="cps1", bufs=3, space="PSUM") as cps1:
            ype = [qsec.tile([128, 4356], bf16, tag=f"ype{kt}", name=f"ype{kt}") for kt in range(2)]
            for kt in range(2):
                nc.sync.dma_start(out=ype[kt][:, 0:2178], in_=ype_d[kt][:, 0:2178])
                nc.sync.dma_start(out=ype[kt][:, 2178:4356], in_=ype_d[kt][:, 2178:4356])
            ype_v = [ype[kt].rearrange("p (r c) -> p r c", c=66) for kt in range(2)]
            qraw = qsec.tile([128, 4096], f32, tag="qraw")
            for chunk in range(8):
                qps = cps1.tile([128, 512], f32, tag="qps", name=f"qps{chunk}")
                r0 = 8 * chunk
                idx = 0
                for t in range(9):
                    ky, kx = t // 3, t % 3
                    for kt in range(2):
                        nc.tensor.matmul(
                            qps[:, :],
                            wq_v[:, t * 2 + kt, :],
                            ype_v[kt][:, r0 + ky : r0 + ky + 8, kx : kx + 64],
                            start=(idx == 0), stop=(idx == 17),
                        )
                        idx += 1
                nc.vector.tensor_copy(qraw[:, 512 * chunk : 512 * chunk + 512], qps[:, :])
                nc.vector.bn_stats(out=qstats[:, chunk, :], in_=qraw[:, 512 * chunk : 512 * chunk + 512])

            qmv = small.tile([128, 2], f32, tag="qmv")
            nc.vector.bn_aggr(out=qmv, in_=qstats)
            qinv = rsqrt_col(qmv[:, 1:2], 128, "qinv")
            qbias = small.tile([128, 1], f32, tag="qbias")
            nc.vector.tensor_scalar(out=qbias, in0=qmv[:, 0:1], scalar1=qinv, scalar2=-1.0,
                                    op0=ALU.mult, op1=ALU.mult)
            nc.scalar.activation(out=qrep, in_=qraw, func=AF.Silu, bias=qbias, scale=qinv)

        # prefetch green inputs + gate during attention (sync queue is idle then)
        yrep = [gin.tile([128, 4356], bf16, tag=f"yrep{kt}", name=f"yrep{kt}") for kt in range(2)]
        for kt in range(2):
            nc.sync.dma_start(out=yrep[kt][:, :], in_=yrep_d[kt][:, :])
        yrep_v = [yrep[kt].rearrange("p (r c) -> p r c", c=66) for kt in range(2)]
        sgate_sb = gin.tile([128, 4096], f32, tag="sgate")
        nc.sync.dma_start(out=sgate_sb, in_=sgate_d[:, :])

        # =========== attention ===========
        # mha_pad [32 ch, 64 rows x 66 cols] (columns edge-padded), bf16
        mha_pad = attn.tile([32, 4224], bf16, tag="mhapad32")
        mha_pad_v = mha_pad.rearrange("p (r c) -> p r c", c=66)

        with tc.tile_pool(name="aexpp", bufs=4) as aexp_pool, \
             tc.tile_pool(name="qkps", bufs=2, space="PSUM") as qkps, \
             tc.tile_pool(name="pvps", bufs=2, space="PSUM") as pvps, \
             tc.tile_pool(name="rsps", bufs=2, space="PSUM") as rsps:
            for I in range(8):
                pvt = pvps.tile([128, 512], f32, tag="pvt", name=f"pvt{I}")
                for g in range(16):
                    qk = qkps.tile([128, 1024], f32, tag="qk", name=f"qk{I}_{g}")
                    for t in range(2):
                        jb = 2 * g + t
                        nc.tensor.matmul(
                            qk[:, 512 * t : 512 * t + 512],
                            qrep[0:32, 128 * jb : 128 * jb + 128],
                            qrep[0:32, 512 * I : 512 * I + 512],
                            start=True, stop=True,
                        )
                    aexp = aexp_pool.tile([128, 1024], bf16, tag="aexp", name=f"ae{I}_{g}")
                    nc.scalar.activation(out=aexp, in_=qk, func=AF.Exp, scale=SCALE)
                    for t in range(2):
                        jb = 2 * g + t
                        nc.tensor.matmul(
                            pvt[0:33, :],
                            vT_v[:, jb, :],
                            aexp[:, 512 * t : 512 * t + 512],
                            start=(g == 0 and t == 0), stop=(g == 15 and t == 1),
                            skip_group_check=True,
                        )
                # per-I epilogue: divide by row sums, stage AllGather payload
                rrow = small.tile([1, 512], f32, tag="rrow", name=f"rrow{I}")
                nc.vector.reciprocal(out=rrow, in_=pvt[32:33, :])
                rsp = rsps.tile([32, 512], f32, tag="rsp", name=f"rsp{I}")
                nc.tensor.matmul(rsp[:, :], ones1[0:1, :], rrow, start=True, stop=True)
                rs_sb = small.tile([32, 512], f32, tag="rs_sb", name=f"rs_sb{I}")
                nc.vector.tensor_copy(rs_sb, rsp)
                dst = mha_pad_v[:, 8 * I : 8 * I + 8, 1:65]
                nc.vector.tensor_mul(
                    dst,
                    pvt[0:32, :].rearrange("p (r c) -> p r c", c=64),
                    rs_sb.rearrange("p (r c) -> p r c", c=64))
                nc.vector.tensor_copy(mha_pad_v[:, 8 * I : 8 * I + 8, 0:1],
                                      mha_pad_v[:, 8 * I : 8 * I + 8, 1:2])
                nc.vector.tensor_copy(mha_pad_v[:, 8 * I : 8 * I + 8, 65:66],
                                      mha_pad_v[:, 8 * I : 8 * I + 8, 64:65])
                nc.gpsimd.dma_start(out=cc1_in[:, 528 * I : 528 * I + 528],
                                    in_=mha_pad[:, 528 * I : 528 * I + 528])

        late = ctx.enter_context(tc.tile_pool(name="late", bufs=1))

        # AllGather mha across the 4 cores of this batch (bf16, padded cols)
        if no_cc:
            for g in range(4):
                nc.gpsimd.dma_start(out=cc1_out[32 * g : 32 * g + 32, :], in_=cc1_in[:, :])
        else:
            nc.gpsimd.collective_compute(
                "AllGather", mybir.AluOpType.bypass,
                replica_groups=_REPLICA_GROUPS,
                ins=[cc1_in[:, :]],
                outs=[cc1_out[:, :]],
            )

        # =========== green conv (overlaps the AllGather) ===========
        greenraw = late.tile([128, 4096], f32, tag="greenraw")
        gstats = small.tile([128, 8, 6], f32, tag="gstats")
        with tc.tile_pool(name="gps", bufs=2, space="PSUM") as gps_pool, \
             tc.tile_pool(name="lps1", bufs=1, space="PSUM") as lps1:
            for chunk in range(8):
                gtile = gps_pool.tile([128, 512], f32, tag="gpsum", name=f"g{chunk}")
                r0 = 8 * chunk
                idx = 0
                for tap in range(9):
                    ey, ex = tap // 3, tap % 3
                    for kt in range(2):
                        nc.tensor.matmul(
                            gtile[:, :],
                            wg_v[:, tap * 2 + kt, :],
                            yrep_v[kt][:, r0 + ey : r0 + ey + 8, ex : ex + 64],
                            start=(idx == 0), stop=(idx == 17),
                        )
                        idx += 1
                col = 512 * chunk
                nc.vector.tensor_copy(greenraw[:, col : col + 512], gtile[:, :])
                nc.vector.bn_stats(out=gstats[:, chunk, :], in_=greenraw[:, col : col + 512])

            gmv = small.tile([128, 2], f32, tag="gmv")
            nc.vector.bn_aggr(out=gmv, in_=gstats)

            def part_sums(mv, tag):
                s2 = small.tile([128, 2], f32, tag=tag, name=tag)
                nc.vector.tensor_scalar_mul(s2[:, 0:1], mv[:, 0:1], 4096.0)
                t = small.tile([128, 1], f32, tag=tag + "t", name=tag + "t")
                nc.vector.tensor_mul(t, mv[:, 0:1], mv[:, 0:1])
                nc.vector.tensor_add(t, t, mv[:, 1:2])
                nc.vector.tensor_scalar_mul(s2[:, 1:2], t, 4096.0)
                return s2

            gsums2 = part_sums(gmv, "gsums2")
            # per-channel sums (combine 4 phases) + layer partial sums
            chps = lps1.tile([128, 512], f32, tag="lps1", name="chps")
            nc.tensor.matmul(chps[0:32, 0:2], sel_sb, gsums2, start=True, stop=True)
            nc.tensor.matmul(chps[0:1, 4:6], ones_sb, gsums2, start=True, stop=True,
                             skip_group_check=True)
            gch = small.tile([32, 2], f32, tag="gch")
            nc.vector.tensor_copy(gch, chps[0:32, 0:2])
            glsb = small.tile([1, 2], f32, tag="glsb")
            nc.vector.tensor_copy(glsb, chps[0:1, 4:6])
            nc.sync.dma_start(out=cc2g_in[:, :], in_=glsb)
            if no_cc:
                nc.gpsimd.dma_start(out=cc2g_out[:, :], in_=cc2g_in[:, :])
            else:
                nc.gpsimd.collective_compute(
                    "AllReduce", mybir.AluOpType.add,
                    replica_groups=_REPLICA_GROUPS,
                    ins=[cc2g_in[:, :]],
                    outs=[cc2g_out[:, :]],
                )

        # mhapad load (scalar queue; waits on the AllGather)
        mhapad = late.tile([128, 4224], bf16, tag="mhapad")
        nc.scalar.dma_start(out=mhapad, in_=cc1_out[:, :])
        mhapad_v = mhapad.rearrange("p (r c) -> p r c", c=66)

        # ---- ILN affines ----
        def iln_affine(ch_sums, S_col, aff_sb, tag):
            n, n1 = N_PX, N_PX - 1.0
            nt, nt1 = N_TOT, N_TOT - 1.0
            in_m = small.tile([32, 1], f32, tag=tag + "im", name=tag + "im")
            nc.vector.tensor_scalar_mul(in_m, ch_sums[:, 0:1], 1.0 / n)
            t1 = small.tile([32, 1], f32, tag=tag + "t1", name=tag + "t1")
            nc.vector.tensor_mul(t1, ch_sums[:, 0:1], ch_sums[:, 0:1])
            nc.vector.tensor_scalar_mul(t1, t1, 1.0 / n)
            nc.vector.tensor_sub(t1, ch_sums[:, 1:2], t1)
            in_v = small.tile([32, 1], f32, tag=tag + "iv", name=tag + "iv")
            nc.vector.tensor_scalar_mul(in_v, t1, 1.0 / n1)
            inv_in = rsqrt_col(in_v, 32, tag + "ii")

            ln_m = small.tile([32, 1], f32, tag=tag + "lm", name=tag + "lm")
            nc.vector.tensor_scalar_mul(ln_m, S_col[:, 0:1], 1.0 / nt)
            l1 = small.tile([32, 1], f32, tag=tag + "l1", name=tag + "l1")
            nc.vector.tensor_mul(l1, S_col[:, 0:1], S_col[:, 0:1])
            nc.vector.tensor_scalar_mul(l1, l1, 1.0 / nt)
            nc.vector.tensor_sub(l1, S_col[:, 1:2], l1)
            ln_v = small.tile([32, 1], f32, tag=tag + "lv", name=tag + "lv")
            nc.vector.tensor_scalar_mul(ln_v, l1, 1.0 / nt1)
            inv_ln = rsqrt_col(ln_v, 32, tag + "il")

            rho = aff_sb[:, 0:1]
            t3 = small.tile([32, 1], f32, tag=tag + "t3", name=tag + "t3")
            nc.vector.tensor_mul(t3, rho, inv_in)
            t6 = small.tile([32, 1], f32, tag=tag + "t6", name=tag + "t6")
            nc.vector.tensor_mul(t6, rho, inv_ln)
            nc.vector.tensor_sub(t6, inv_ln, t6)
            A = small.tile([32, 1], f32, tag=tag + "A", name=tag + "A")
            nc.vector.tensor_add(A, t3, t6)
            u1 = small.tile([32, 1], f32, tag=tag + "u1", name=tag + "u1")
            nc.vector.tensor_mul(u1, in_m, t3)
            u2 = small.tile([32, 1], f32, tag=tag + "u2", name=tag + "u2")
            nc.vector.tensor_mul(u2, ln_m, t6)
            nc.vector.tensor_add(u1, u1, u2)
            B = small.tile([32, 1], f32, tag=tag + "B", name=tag + "B")
            nc.vector.tensor_scalar_mul(B, u1, -1.0)
            sb = small.tile([32, 2], f32, tag=tag + "sb", name=tag + "sb")
            nc.vector.tensor_mul(sb[:, 0:1], A, aff_sb[:, 1:2])
            nc.vector.tensor_mul(sb[:, 1:2], B, aff_sb[:, 1:2])
            nc.vector.tensor_add(sb[:, 1:2], sb[:, 1:2], aff_sb[:, 2:3])
            return sb

        # =========== purple conv + tails ===========
        with tc.tile_pool(name="gps2", bufs=2, space="PSUM") as gps2, \
             tc.tile_pool(name="tailps", bufs=2, space="PSUM") as tailps:
            purpleraw = late.tile([128, 4096], f32, tag="purpleraw")
            pstats = small.tile([128, 8, 6], f32, tag="pstats")
            for chunk in range(8):
                ptile = gps2.tile([128, 512], f32, tag="gpsum2", name=f"pt{chunk}")
                pt_v = ptile.rearrange("p (r c) -> p r c", c=64)
                r0 = 8 * chunk
                # tap order: start with ey=1 (always fully in range)
                order = [3, 4, 5, 0, 1, 2, 6, 7, 8]
                for n_i, tap in enumerate(order):
                    ey, ex = tap // 3, tap % 3
                    rlo = r0 + ey - 1
                    first = (n_i == 0)
                    last = (n_i == 8)
                    if rlo >= 0 and rlo + 7 <= 63:
                        nc.tensor.matmul(
                            ptile[:, :],
                            wp_v[:, tap, :],
                            mhapad_v[:, rlo : rlo + 8, ex : ex + 64],
                            start=first, stop=last, skip_group_check=True,
                        )
                    elif rlo < 0:
                        # chunk 0, ey=0: rows 1..7 read rows 0..6; row 0 clamps to row 0
                        nc.tensor.matmul(
                            pt_v[:, 1:8, :],
                            wp_v[:, tap, :],
                            mhapad_v[:, 0:7, ex : ex + 64],
                            start=False, stop=False, skip_group_check=True,
                        )
                        nc.tensor.matmul(
                            pt_v[:, 0:1, :],
                            wp_v[:, tap, :],
                            mhapad_v[:, 0:1, ex : ex + 64],
                            start=False, stop=last, skip_group_check=True,
                        )
                    else:
                        # chunk 7, ey=2: rows 0..6 read rows 57..63; row 7 clamps to 63
                        nc.tensor.matmul(
                            pt_v[:, 0:7, :],
                            wp_v[:, tap, :],
                            mhapad_v[:, 57:64, ex : ex + 64],
                            start=False, stop=False, skip_group_check=True,
                        )
                        nc.tensor.matmul(
                            pt_v[:, 7:8, :],
                            wp_v[:, tap, :],
                            mhapad_v[:, 63:64, ex : ex + 64],
                            start=False, stop=last, skip_group_check=True,
                        )
                col = 512 * chunk
                nc.vector.tensor_copy(purpleraw[:, col : col + 512], ptile[:, :])
                nc.vector.bn_stats(out=pstats[:, chunk, :], in_=purpleraw[:, col : col + 512])
            pmv = small.tile([128, 2], f32, tag="pmv")
            nc.vector.bn_aggr(out=pmv, in_=pstats)

            psums2 = part_sums(pmv, "psums2")
            chps2 = tailps.tile([128, 512], f32, tag="tps", name="chps2")
            nc.tensor.matmul(chps2[0:32, 0:2], sel_sb, psums2, start=True, stop=True)
            nc.tensor.matmul(chps2[0:1, 4:6], ones_sb, psums2, start=True, stop=True,
                             skip_group_check=True)
            pch = small.tile([32, 2], f32, tag="pch")
            nc.vector.tensor_copy(pch, chps2[0:32, 0:2])
            plsb = small.tile([1, 2], f32, tag="plsb")
            nc.vector.tensor_copy(plsb, chps2[0:1, 4:6])
            nc.sync.dma_start(out=cc2p_in[:, :], in_=plsb)
            if no_cc:
                nc.gpsimd.dma_start(out=cc2p_out[:, :], in_=cc2p_in[:, :])
            else:
                nc.gpsimd.collective_compute(
                    "AllReduce", mybir.AluOpType.add,
                    replica_groups=_REPLICA_GROUPS,
                    ins=[cc2p_in[:, :]],
                    outs=[cc2p_out[:, :]],
                )

            # ---- green finalize (overlaps purple stats CC) ----
            lngg = small.tile([32, 2], f32, tag="lngg")
            nc.sync.dma_start(out=lngg, in_=bass.AP(tensor=cc2g_out, offset=0, ap=[[0, 32], [1, 2]]))
            gsb = iln_affine(gch, lngg, affg_sb, "ga")
            gbps = tailps.tile([128, 512], f32, tag="tps", name="gbps")
            nc.tensor.matmul(gbps[:, 0:2], selT_sb, gsb, start=True, stop=True)
            gsb128 = small.tile([128, 2], f32, tag="gsb128")
            nc.vector.tensor_copy(gsb128, gbps[:, 0:2])
            upy_sb = late.tile([128, 4096], f32, tag="upy")
            nc.scalar.activation(out=upy_sb, in_=greenraw, func=AF.Silu,
                                 bias=gsb128[:, 1:2], scale=gsb128[:, 0:1])
            nc.sync.dma_start(out=upyout_d[:, :], in_=upy_sb)

            # ---- purple finalize ----
            lngp = small.tile([32, 2], f32, tag="lngp")
            nc.sync.dma_start(out=lngp, in_=bass.AP(tensor=cc2p_out, offset=0, ap=[[0, 32], [1, 2]]))
            psb = iln_affine(pch, lngp, affp_sb, "pa")
            pbps = tailps.tile([128, 512], f32, tag="tps", name="pbps")
            nc.tensor.matmul(pbps[:, 0:2], selT_sb, psb, start=True, stop=True)
            psb128 = small.tile([128, 2], f32, tag="psb128")
            nc.vector.tensor_copy(psb128, pbps[:, 0:2])

            zpre = late.tile([128, 4096], f32, tag="zpre")
            for half in range(2):
                cols = slice(2048 * half, 2048 * half + 2048)
                nc.scalar.activation(out=zpre[:, cols], in_=purpleraw[:, cols],
                                     func=AF.Sigmoid,
                                     bias=psb128[:, 1:2], scale=psb128[:, 0:1])
                nc.vector.tensor_mul(zpre[:, cols], zpre[:, cols], sgate_sb[:, cols])
                nc.sync.dma_start(out=zout_d[:, cols], in_=zpre[:, cols])

    nc.compile()
    return nc


_NC_CACHE = None
RUN_KWARGS = {}      # test harness may set e.g. {"trace": True}
LAST_RESULTS = None  # BassKernelResults of the most recent run


def kernel(**inputs) -> np.ndarray:
    global _NC_CACHE, LAST_RESULTS
    from concourse.bass_utils import run_bass_kernel_spmd

    if _NC_CACHE is None:
        _NC_CACHE = build_bass()
    nc = _NC_CACHE

    in_maps = []
    for core in _CORES:
        ci = prepare_core_inputs(inputs, core)
        in_maps.append(ci)

    res = run_bass_kernel_spmd(nc, in_maps, _CORES, **RUN_KWARGS)
    LAST_RESULTS = res
    zs = [res.results[c]["zout"] for c in _CORES]
    upys = [res.results[c]["upyout"] for c in _CORES]
    return assemble_output(zs, upys)


if __name__ == "__main__":
    nc = build_bass()
    print("bass build OK")
